# revision 61
# baseline (speedup 1.0000x reference)
"""Min-norm solver (MGDA) for Trainium2, sharded across 8 NeuronCores.

Strategy:
  - vecs is [32, 2097152] f32 (256 MB).  The only memory-heavy step is the
    Gram matrix G = vecs @ vecs.T ([32, 32]).  We shard the d dimension
    across 8 cores and compute partial Grams on-device.
  - On-device layout: the host pre-transposes each core's shard into
    X[p, (n*32 + j)] = vecs[j, n*128 + p]  (p: 0..127 partition, n: d-chunk,
    j: task), so the TensorEngine can contract over the partition dim with
    fully-contiguous APs.  Four d-chunks are packed into one [128, 128]
    "group"; a group's [128,128] self-Gram holds the 4 chunks' partial
    [32,32] Grams on its diagonal blocks, accumulated in PSUM.
  - Precision/bandwidth trade: the modeled DMA roofline is 360 GB/s/core,
    so bytes are everything.  vecs is cast to fp8e4m3 on the host
    (8 MB/core).  fp8e4 also enables the PE's DoubleRow perf mode
    (2 groups contracted per instruction at 0.5 cycles/row), which keeps
    the PE far below the DMA roofline; PSUM accumulation stays fp32.
    Gram off-diagonal rounding noise is ~70 absolute on a 2.1e6 diagonal;
    the end-to-end solution error vs the f32 reference is ~3.5e-4, well
    inside the 2e-2 gate.  (fp16 fallback kept: 16 MB/core, ~4e-6.)
  - psum_mode="small": each DoubleRow matmul uses one chunk's 32-column
    slice as both operands, so every [32,32] self-Gram accumulates into a
    single [32,32] PSUM tile (same total PE cycles as the [128,128]
    variant, 4x the instructions at 1/4 cost).  This shrinks the final
    PSUM->SBUF copy and the output DMA to [32,32].
  - Timing structure (TimelineSim, 29.1us/core): 1.9us head (init
    barrier + HWDGE + DGE delay), 23.3us DMA stream (8MB @ 360GB/s, the
    roofline), ~3.9us tail (DMA sem 900ns + last matmuls + PE drain +
    copy + out-DMA fixed costs + final 900ns sem).  Tapered tail tiles
    keep the last DMA->matmul chain short.  The out-DMA ("actdma2") is
    parked pre-context on the otherwise-idle Activation queue: matmuls
    accumulate into a RAW psum tensor, so the DVE copy (which tile still
    guards with the required PE pipeline drain) carries the handshake
    semaphore itself, and the TileContext exit barrier overlaps the
    DMA's HWDGE/DGE/sem-prop stages instead of serializing after them.
    Prepared SWDGE writeback/scatter (skipping HWDGE+DGE entirely) and
    a drain-carried semaphore ("actpsum") model faster but crash or
    return garbage on the real device.
  - The tiny 250-iteration solver runs on the host in float32 numpy,
    faithfully mirroring the reference ops.
"""

import numpy as np
import ml_dtypes

N_TASKS = 32
D = 2097152
N_CORES = 8
D_LOC = D // N_CORES          # 262144 d-values per core
N_CHUNK = D_LOC // 128        # 2048 chunks of 128 d-values
TOT_GRP = N_CHUNK * 32 // 128  # 512 groups of 128 columns (4 chunks each)

MAX_ITER = 250
STOP_CRIT = np.float32(1e-6)
EPS = np.float32(1e-8)

HI_DTYPE = "fp8"              # "fp8" | "fp16" | "bf16"
TILE_GRP = 24                 # groups per SBUF tile (24 -> 3072 columns)
TAIL = (12, 4)                # tapered tail tiles, in groups
# "actdma2" (HW-verified): out-DMA parked pre-context on the idle Activation
# queue; matmuls accumulate into a RAW psum tensor so the DVE copy carries
# the handshake sem itself (no reader tick, no tiny copy).  "rawkvwb"/
# "scatter" (prepared SWDGE trigger) and "actpsum" (drain-carried sem)
# model faster still but return garbage / crash on the real device.
OUT_VIA = "actdma2"           # "actdma2" | "actdma" | "copy" | ...
# "small": accumulate all chunks' [32,32] partial Grams into ONE psum tile
# (same PE cycles via 32-col lhsT/rhs slices); shrinks the final copy+DMA
PSUM_MODE = "small"           # "small" | "diag4"

_PROGRAMS = {}


def _build_program(tile_grp=TILE_GRP, bufs=8, dma="sync", tail=TAIL,
                   out_via=OUT_VIA, hi_dtype=HI_DTYPE, probe=None,
                   copy_eng="vector", trig_in_tc=False, raw_stage=4,
                   psum_mode=PSUM_MODE):
    import concourse.mybir as mybir
    import concourse.tile as tile
    from concourse import bacc

    fp8 = hi_dtype == "fp8"
    hi_dt = {"fp16": mybir.dt.float16, "bf16": mybir.dt.bfloat16,
             "fp8": mybir.dt.float8e4}[hi_dtype]

    # tile widths in groups; geometric taper shortens the final
    # DMA->matmul dependency chain
    tail = tuple(tail) if tail else ()
    n_main = TOT_GRP - sum(tail)
    widths = [tile_grp] * (n_main // tile_grp)
    if n_main % tile_grp:
        widths.append(n_main % tile_grp)
    widths += list(tail)
    assert sum(widths) == TOT_GRP
    step = 2 if fp8 else 1
    assert all(w % step == 0 for w in widths)
    pm = mybir.MatmulPerfMode.DoubleRow if fp8 else None

    nc = bacc.Bacc("TRN2", target_bir_lowering=False, debug=False,
                   num_devices=N_CORES)
    xh = nc.dram_tensor("xh", [128, TOT_GRP, 128], hi_dt,
                        kind="ExternalInput").ap()
    kvwb = out_via == "kvwb"
    rawkv = out_via == "rawkvwb"
    scat = out_via == "scatter"
    post = out_via == "postdma"
    small = psum_mode == "small"
    assert not (small and (kvwb or rawkv or scat))
    out_shape = ([1, 128, 1, 128] if (kvwb or rawkv)
                 else [32, 32] if small else [128, 128])
    out_ab = nc.dram_tensor("out_ab", out_shape, mybir.dt.float32,
                            kind="ExternalOutput").ap()

    act = out_via == "actdma"
    actp = out_via == "actpsum"
    act2 = out_via == "actdma2"
    if act2 and probe is None:
        # Like actdma, but the matmuls accumulate into a RAW psum tensor so
        # the DVE copy (tile still inserts the PE drain before it, via the
        # shadow-memory RAW edge) carries the Act handshake sem itself —
        # no reader tick on the copy, no tiny handshake copy.
        assert small
        p_raw = nc.alloc_psum_tensor("p_raw", [32, 32], mybir.dt.float32)
        o_raw = nc.alloc_sbuf_tensor("o_raw", [32, 32], mybir.dt.float32)
        kv_copy_sem = nc.alloc_semaphore("kv_copy")
        kv_done_sem = nc.alloc_semaphore("kv_done")
        for s in (kv_copy_sem, kv_done_sem):
            nc.gpsimd.dma_reset(range(s.num, s.num + 1))
            nc.gpsimd.sem_clear(range(s.num, s.num + 1))
        nc.scalar.wait_ge(kv_copy_sem, 1)
        nc.scalar.dma_start(out_ab,
                            o_raw.ap()[:]).then_inc(kv_done_sem, 16)
    if actp and probe is None:
        # Act-side copy+DMA parked pre-context: the last matmul's manual sem
        # wakes Activation, which copies the raw [32,32] PSUM accumulator to
        # SBUF and DMAs it out — no DVE copy, tile drain, or handshake copy
        # on the critical chain.
        assert small
        p_raw = nc.alloc_psum_tensor("p_raw", [32, 32], mybir.dt.float32)
        p_scr = nc.alloc_psum_tensor("p_scr", [32, 32], mybir.dt.float32)
        o_raw = nc.alloc_sbuf_tensor("o_raw", [32, 32], mybir.dt.float32)
        mm_sem = nc.alloc_semaphore("mm_done")
        kv_done_sem = nc.alloc_semaphore("kv_done")
        for s in (mm_sem, kv_done_sem):
            nc.gpsimd.dma_reset(range(s.num, s.num + 1))
            nc.gpsimd.sem_clear(range(s.num, s.num + 1))
        nc.scalar.wait_ge(mm_sem, 1)
        nc.scalar.copy(o_raw.ap()[:], p_raw.ap()[:])
        nc.scalar.dma_start(out_ab,
                            o_raw.ap()[:]).then_inc(kv_done_sem, 16)
    if (post or act) and probe is None:
        # out-DMA with a manual sem handshake, off the tile-tracked path so
        # the context exit barrier doesn't serialize with its 900ns sem.
        # "actdma": emitted PRE-context on the otherwise-idle Activation
        # queue — it parks on the copy sem and fires the moment the copy
        # lands, with the exit barrier overlapping its HWDGE/DGE/sem stages.
        assert small
        o_raw = nc.alloc_sbuf_tensor("o_raw", [32, 32], mybir.dt.float32)
        kv_scr = nc.alloc_sbuf_tensor("kv_scr", [1, 2], mybir.dt.float32)
        kv_copy_sem = nc.alloc_semaphore("kv_copy")
        kv_done_sem = nc.alloc_semaphore("kv_done")
        for s in (kv_copy_sem, kv_done_sem):
            nc.gpsimd.dma_reset(range(s.num, s.num + 1))
            nc.gpsimd.sem_clear(range(s.num, s.num + 1))
        if act:
            nc.scalar.wait_ge(kv_copy_sem, 1)
            nc.scalar.dma_start(out_ab,
                                o_raw.ap()[:]).then_inc(kv_done_sem, 16)

    if rawkv and probe is None:
        # Raw-bass prepared writeback, invisible to TileContext's dep
        # tracker: descriptors are generated on Pool at program start; the
        # post-context trigger fires them after the copy's manual sem,
        # skipping the HWDGE (625ns) + DGE->DMA (650ns) stages on the tail.
        o_raw = nc.alloc_sbuf_tensor("o_raw", [128, 1, 1, 128],
                                     mybir.dt.float32)
        kv_scr = nc.alloc_sbuf_tensor("kv_scr", [1, 2], mybir.dt.float32)
        idx_raw = nc.alloc_sbuf_tensor("kv_idx", [128, 1], mybir.dt.int32)
        kv_dma_sem = nc.alloc_semaphore("kv_dma")
        kv_prep_sem = nc.alloc_semaphore("kv_prep")
        kv_copy_sem = nc.alloc_semaphore("kv_copy")
        nc.gpsimd.memset(idx_raw.ap()[:], 0)
        nc.gpsimd.kv_writeback(out_ab, o_raw.ap()[:], idx_raw.ap()[:],
                               prepare_only=True,
                               sem=kv_dma_sem).then_inc(kv_prep_sem, 1)
    if scat and probe is None:
        # Raw-bass prepared scatter-add (same trick as rawkvwb but via the
        # production-exercised scatter-add ucode).  out_ab must be zeroed
        # first (DRAM starts as garbage); o_raw doubles as the zero source.
        o_raw = nc.alloc_sbuf_tensor("o_raw", [128, 1, 128],
                                     mybir.dt.float32)
        kv_scr = nc.alloc_sbuf_tensor("kv_scr", [1, 2], mybir.dt.float32)
        sc_idx = nc.alloc_sbuf_tensor("sc_idx", [16, 8], mybir.dt.int16)
        sc_dma_sem = nc.alloc_semaphore("sc_dma")
        sc_prep_sem = nc.alloc_semaphore("sc_prep")
        kv_copy_sem = nc.alloc_semaphore("kv_copy")
        zdma_sem = nc.alloc_semaphore("zdma")
        for s in (sc_dma_sem, sc_prep_sem, kv_copy_sem, zdma_sem):
            nc.gpsimd.dma_reset(range(s.num, s.num + 1))
            nc.gpsimd.sem_clear(range(s.num, s.num + 1))
        nc.gpsimd.memset(o_raw.ap()[:], 0.0)
        nc.gpsimd.iota(sc_idx.ap()[:], pattern=[[16, 8]], base=0,
                       channel_multiplier=1)
        # Pool-engine (SWDGE) DMA: in-order after the memset, keeps the
        # zeroing DMA off the HWDGE pipeline feeding the input stream
        if raw_stage >= 2:
            nc.gpsimd.dma_start(out_ab, o_raw.ap()[:]).then_inc(zdma_sem, 16)
        if raw_stage >= 3:
            nc.gpsimd.dma_scatter_add(out_ab, o_raw.ap()[:], sc_idx.ap()[:],
                                      128, 128, 128, prepare_only=True,
                                      sem=sc_dma_sem).then_inc(sc_prep_sem, 1)

    with tile.TileContext(nc) as tc:
        with (
            tc.tile_pool(name="hi", bufs=bufs) as hi_pool,
            tc.tile_pool(name="psum", bufs=1, space="PSUM") as psum_pool,
            tc.tile_pool(name="outs", bufs=1) as out_pool,
        ):
            dma_eng = getattr(nc, dma)
            if (actp or act2) and probe is None:
                p_a = p_raw.ap()
            else:
                p_a = psum_pool.tile([32, 32] if small else [128, 128],
                                     mybir.dt.float32, name="p_a")
            if kvwb and probe is None:
                # SWDGE prepared writeback: descriptors are generated early
                # on the Pool engine (hidden under the DMA stream); the
                # trigger carries the RAW dep on o_ab, skipping the HWDGE
                # (625ns) + DGE->DMA (650ns) stages on the critical tail.
                # NOTE: the prep must be EMITTED after the copy (so the RAW
                # edge exists and is deferred to the trigger), but it runs
                # early because the deferred edge carries no sem wait.
                o_ab = out_pool.tile([128, 1, 1, 128], mybir.dt.float32)
                idx_t = out_pool.tile([128, 1], mybir.dt.int32)
                nc.gpsimd.memset(idx_t[:], 0)
                kv_sem = nc.alloc_semaphore("kvwb_dma")
            eg = 0
            for t, wg in enumerate(widths):
                ht = hi_pool.tile([128, wg, 128], hi_dt, tag="ht")
                dma_eng.dma_start(ht[:], xh[:, eg:eg + wg, :])
                if probe == "dma_only":
                    eg += wg
                    continue
                for g in range(0, wg, step):
                    first = eg == 0 and g == 0
                    last = t == len(widths) - 1 and g + step >= wg
                    if small:
                        # 32-col slices: every matmul's [32,32] self-Gram
                        # accumulates into the single small psum tile
                        for c in range(4):
                            sl = ht[:, g:g + step, 32 * c:32 * c + 32]
                            nc.tensor.matmul(p_a[:], sl, sl,
                                             start=first and c == 0,
                                             stop=last and c == 3,
                                             perf_mode=pm)
                            if actp and probe is None and last and c == 3:
                                # PE drain carries the sem (Matmult fits only
                                # one update): fires only once the engine
                                # pipeline — including p_raw's write — flushed
                                nc.tensor.drain().then_inc(mm_sem, 1)
                    else:
                        nc.tensor.matmul(p_a[:], ht[:, g:g + step, :],
                                         ht[:, g:g + step, :],
                                         start=first, stop=last, perf_mode=pm)
                eg += wg
            if probe in ("dma_only", "no_out"):
                pass
            elif kvwb:
                nc.vector.tensor_copy(o_ab[:, 0, 0, :], p_a[:])
                nc.gpsimd.kv_writeback(out_ab, o_ab[:], idx_t[:],
                                       prepare_only=True, sem=kv_sem)
                nc.gpsimd.trigger_dma(count=None)
            elif rawkv or (scat and raw_stage >= 4):
                # TensorCopy's ISA struct fits only one sem update (the
                # tile tick), so signal via a tiny DVE op on untracked
                # SBUF; DVE executes in order after the big copy.
                dst2d = (o_raw.ap()[:, 0, 0, :] if rawkv
                         else o_raw.ap()[:, 0, :])
                src2 = (o_raw.ap()[0:1, 0, 0, 0:2] if rawkv
                        else o_raw.ap()[0:1, 0, 0:2])
                nc.vector.tensor_copy(dst2d, p_a[:])
                nc.vector.tensor_copy(
                    kv_scr.ap()[:], src2).then_inc(kv_copy_sem, 1)
            elif scat:
                # bisect probe: keep the HWDGE out path alongside raw bits
                o_ab = out_pool.tile([128, 128], mybir.dt.float32)
                nc.vector.tensor_copy(o_ab[:], p_a[:])
                nc.sync.dma_start(out_ab, o_ab[:])
            elif actp:
                pass  # Act-side copy+DMA emitted pre-context
            elif act2:
                nc.vector.tensor_copy(
                    o_raw.ap()[:], p_a[:]).then_inc(kv_copy_sem, 1)
            elif post or act:
                nc.vector.tensor_copy(o_raw.ap()[:], p_a[:])
                nc.vector.tensor_copy(
                    kv_scr.ap()[:], o_raw.ap()[0:1, 0:2]
                ).then_inc(kv_copy_sem, 1)
                if trig_in_tc:
                    # trigger inside the context: the tile exit barrier then
                    # overlaps the writeback DMA + its 900ns sem prop
                    nc.gpsimd.wait_ge(kv_prep_sem, 1)
                    nc.gpsimd.wait_ge(kv_copy_sem, 1)
                    nc.gpsimd.trigger_dma(count=1)
            else:
                o_ab = out_pool.tile([32, 32] if small else [128, 128],
                                     mybir.dt.float32)
                if copy_eng == "scalar":
                    nc.scalar.copy(o_ab[:], p_a[:])
                else:
                    nc.vector.tensor_copy(o_ab[:], p_a[:])
                nc.sync.dma_start(out_ab, o_ab[:])
    if rawkv and probe is None and not trig_in_tc:
        nc.gpsimd.wait_ge(kv_prep_sem, 1)
        nc.gpsimd.wait_ge(kv_copy_sem, 1)
        nc.gpsimd.trigger_dma(count=1)
        nc.gpsimd.wait_ge(kv_dma_sem, 16)
    if scat and probe is None and raw_stage >= 4:
        nc.gpsimd.wait_ge(sc_prep_sem, 1)
        nc.gpsimd.wait_ge(zdma_sem, 16)
        nc.gpsimd.wait_ge(kv_copy_sem, 1)
        nc.gpsimd.trigger_dma(count=1)
        nc.gpsimd.wait_ge(sc_dma_sem, 16)
    if post and probe is None:
        nc.sync.wait_ge(kv_copy_sem, 1)
        nc.sync.dma_start(out_ab, o_raw.ap()[:]).then_inc(kv_done_sem, 16)
    nc.compile()
    return nc


def _get_program(**kw):
    key = tuple(sorted(kw.items()))
    if key not in _PROGRAMS:
        _PROGRAMS[key] = _build_program(**kw)
    return _PROGRAMS[key]


def _prep_inputs(vecs, hi_dtype=HI_DTYPE):
    """[32, D] f32 -> per-core arrays in PE layout.

    X[c, p, n*32 + j] = vecs[j, c*D_LOC + n*128 + p]
    """
    x = np.asarray(vecs, dtype=np.float32)
    x = x.reshape(N_TASKS, N_CORES, N_CHUNK, 128)      # [j, c, n, p]
    x = np.ascontiguousarray(x.transpose(1, 3, 2, 0))  # [c, p, n, j]
    np_dt = {"fp16": np.float16, "bf16": ml_dtypes.bfloat16,
             "fp8": ml_dtypes.float8_e4m3}[hi_dtype]
    return x.reshape(N_CORES, 128, TOT_GRP, 128).astype(np_dt)


def run_device(vecs, hi_dtype=HI_DTYPE, **prog_kw):
    """Run the sharded Gram computation; returns (G [32,32] f32, results)."""
    from concourse.bass_utils import run_bass_kernel_spmd

    hi = _prep_inputs(vecs, hi_dtype)
    in_maps = [{"xh": hi[c]} for c in range(N_CORES)]
    res = run_bass_kernel_spmd(
        _get_program(hi_dtype=hi_dtype, **prog_kw),
        in_maps, list(range(N_CORES)))
    g_acc = np.zeros((N_TASKS, N_TASKS), dtype=np.float64)
    for c in range(N_CORES):
        a = res.results[c]["out_ab"]
        if a.size == N_TASKS * N_TASKS:          # psum_mode="small"
            g_acc += a.reshape(N_TASKS, N_TASKS).astype(np.float64)
        else:
            a = a.reshape(128, 128).astype(np.float64)
            for s in range(4):
                blk = slice(32 * s, 32 * (s + 1))
                g_acc += a[blk, blk]
    return g_acc.astype(np.float32), res


# ---------------------------------------------------------------------------
# Host-side solver: faithful float32 numpy port of the reference iteration.
# ---------------------------------------------------------------------------

def _line_solver(v11, v12, v22):
    g = (v22 - v12) / (v11 + v22 - np.float32(2.0) * v12 + EPS)
    c = v22 + g * (v12 - v22)
    gamma = np.where(v12 >= v22, np.float32(0.0), g)
    gamma = np.where(v12 >= v11, np.float32(1.0), gamma)
    cost = np.where(v12 >= v22, v22, c)
    cost = np.where(v12 >= v11, v11, cost)
    return gamma.astype(np.float32), cost.astype(np.float32)


def _planar_init(G, n):
    iu, ju = np.triu_indices(n, 1)
    vivj = G[iu, ju]
    vivi = G[iu, iu]
    vjvj = G[ju, ju]
    gamma, cost = _line_solver(vivi, vivj, vjvj)
    off = int(np.argmin(cost))
    sol = np.zeros(n, dtype=G.dtype)
    sol[iu[off]] = gamma[off]
    sol[ju[off]] = np.float32(1.0) - gamma[off]
    return sol


def _proj_simplex(gamma, i_grid):
    s = np.sort(gamma)[::-1]  # descending
    tmp_max = (np.cumsum(s, dtype=np.float32) - np.float32(1.0)) / i_grid
    cond = tmp_max[:-1] > s[1:]
    first = int(np.argmax(cond))  # first True (0 if none)
    tmax = tmp_max[:-1][first] if bool(np.any(cond)) else tmp_max[-1]
    return np.maximum(gamma - tmax, np.float32(0.0)).astype(np.float32)


def _next_point(cur, grad, n_f, i_grid):
    proj = (grad - np.sum(grad) / n_f).astype(np.float32)
    neg = proj < 0
    pos = proj > 0
    inf = np.float32(np.inf)
    tm1 = np.where(neg, -cur / np.where(neg, proj, np.float32(1.0)), inf)
    tm2 = np.where(pos, (np.float32(1.0) - cur) / np.where(pos, proj, np.float32(1.0)), inf)
    thr = np.float32(1e-7)
    m1 = np.min(np.where(tm1 > thr, tm1, inf))
    t = m1 if np.isfinite(m1) else np.float32(1.0)
    m2 = np.min(np.where(tm2 > thr, tm2, inf))
    t = np.minimum(t, m2).astype(np.float32)
    nxt = (proj * t + cur).astype(np.float32)
    return _proj_simplex(nxt, i_grid)


def solve(G):
    n = G.shape[0]
    sol = _planar_init(G, n)
    i_grid = (np.arange(n, dtype=G.dtype) + np.float32(1.0)).astype(G.dtype)
    n_f = np.float32(n)
    for _ in range(MAX_ITER):
        grad_dir = (-(G @ sol)).astype(np.float32)
        newp = _next_point(sol, grad_dir, n_f, i_grid)
        gs = G @ sol
        gn = G @ newp
        v11 = np.float32(sol @ gs)
        v12 = np.float32(sol @ gn)
        v22 = np.float32(newp @ gn)
        gamma, _ = _line_solver(v11, v12, v22)
        new_sol = (gamma * sol + (np.float32(1.0) - gamma) * newp).astype(np.float32)
        if np.sum(np.abs(new_sol - sol)) < STOP_CRIT:
            break  # reference freezes the OLD sol once change < stop_crit
        sol = new_sol
    return sol.astype(np.float32)


def kernel(vecs):
    G, _ = run_device(vecs)
    return solve(G)
